# revision 1
# baseline (speedup 1.0000x reference)
"""Distributed MHA kernel for one TRN2 chip (8 NeuronCores), Bass/Tile.

Problem: B=4, S=2048, D=1024, H=16 full multi-head attention
(qkv proj -> scaled dot product softmax attention -> o proj).

Sharding (no collectives): core c handles batch b=c//2 and query-token
half c%2 (1024 query tokens).  Each core recomputes K/V projections for
the full 2048 tokens of its batch (+25% PE work, zero cross-core sync).
The host permutes x[b] so the core's query tokens come first; softmax
over keys is permutation invariant, so K/V token order doesn't matter.

On-chip dataflow (per core), all fp32 storage, float32r matmuls:
  x^T [D,S] din-major  -> K^T [dout,tok] head-major   (ACT bias fused)
                       -> V   [tok,dv]   token-major, 65-col head blocks
                          with a ones column (softmax denominator trick)
  per (head, q512): logits^T [k,q] = K_h^T.T @ Q_h^T   (contract hd=64)
                    P^T = exp(0.125 * logits^T)         (ACT, no max sub:
                      logits ~ N(0,1) here, exp is safe in fp32)
                    PV: vals^T[d,q] += V_aug[k,65].T @ P^T[k,q]
                      row 64 of vals^T psum = sum_k P^T = softmax denom
                    normalize by broadcast reciprocal, assemble vals^T
  o proj: out[tok,e] = vals^T[:,tok].T @ o_w^T[:,e]    (DVE bias fused)
"""

import numpy as np

_NC_CACHE = {}


def _build_nc(S, D, H, SQ, use_bf16=True):
    import concourse.bass as bass
    import concourse.mybir as mybir
    import concourse.tile as tile
    from concourse import bacc
    from concourse.bass import ts

    f32 = mybir.dt.float32
    cdt = mybir.dt.bfloat16 if use_bf16 else f32
    Copy = mybir.ActivationFunctionType.Copy
    Exp = mybir.ActivationFunctionType.Exp
    add = mybir.AluOpType.add
    mult = mybir.AluOpType.mult

    P = 128
    hd = D // H            # 64 head dim
    hd1 = hd + 1           # 65: V block + ones column
    ND = D // P            # 8 din/dout chunks
    NT = S // 512          # 4 tok512 chunks (K/V)
    NQ = SQ // 512         # 2 q512 chunks
    NK = S // P            # 16 k-token chunks
    HPC = P // hd          # 2 heads per 128-partition chunk
    NG = D // 512          # 2 dv512 groups
    scale = 1.0 / float(np.sqrt(hd))

    nc = bacc.Bacc(trn_type="TRN2", debug=False)

    xT = nc.declare_dram_parameter("xT", [D, S], cdt, isOutput=False)
    wqT = nc.declare_dram_parameter("wqT", [D, D], cdt, isOutput=False)
    wkT = nc.declare_dram_parameter("wkT", [D, D], cdt, isOutput=False)
    wvT = nc.declare_dram_parameter("wvT", [D, D], cdt, isOutput=False)
    owT = nc.declare_dram_parameter("owT", [D, D], cdt, isOutput=False)
    bq = nc.declare_dram_parameter("bq", [D], f32, isOutput=False)
    bk = nc.declare_dram_parameter("bk", [D], f32, isOutput=False)
    bv = nc.declare_dram_parameter("bv", [D], f32, isOutput=False)
    bo = nc.declare_dram_parameter("bo", [D], f32, isOutput=False)
    out = nc.declare_dram_parameter("out", [SQ, D], f32, isOutput=True)

    # [din, tok] viewed as [p, din_chunk, tok]
    xT_r = xT.ap().rearrange("(c p) s -> p c s", p=P)
    wqT_r = wqT.ap().rearrange("(c p) e -> p c e", p=P)
    wkT_r = wkT.ap().rearrange("(c p) e -> p c e", p=P)
    wvT_r = wvT.ap().rearrange("(c p) e -> p c e", p=P)
    owT_r = owT.ap().rearrange("(c p) e -> p c e", p=P)

    def mm(ps, lhsT, rhs, start, stop):
        nc.tensor.matmul(ps, lhsT, rhs, start=start, stop=stop)

    with tile.TileContext(nc) as tc:
        with (
            tc.tile_pool(name="const", bufs=1) as constp,
            tc.tile_pool(name="kpool", bufs=1) as kpool,
            tc.tile_pool(name="vpool", bufs=1) as vpool,
            tc.tile_pool(name="xpool", bufs=4) as xpool,
            tc.tile_pool(name="wpool", bufs=4) as wpool,
            tc.tile_pool(name="wgpool", bufs=2) as wgpool,
            tc.tile_pool(name="qpool", bufs=2) as qpool,
            tc.tile_pool(name="valspool", bufs=2) as valspool,
            tc.tile_pool(name="ptpool", bufs=4) as ptpool,
            tc.tile_pool(name="opool", bufs=3) as opool,
            tc.tile_pool(name="lpool", bufs=2) as lpool,
            tc.tile_pool(name="lgps", bufs=3, space="PSUM") as lgps,
            tc.tile_pool(name="mmps", bufs=2, space="PSUM") as mmps,
        ):
            # ---- constants: biases ----
            bqs = constp.tile([P, ND], f32)
            nc.sync.dma_start(bqs[:], bq.ap().rearrange("(c p) -> p c", p=P))
            bks = constp.tile([P, ND], f32)
            nc.sync.dma_start(bks[:], bk.ap().rearrange("(c p) -> p c", p=P))
            bvb = constp.tile([P, D], f32)
            nc.sync.dma_start(bvb[:], bv.ap().unsqueeze(0).to_broadcast((P, D)))
            bob = constp.tile([P, D], f32)
            nc.sync.dma_start(bob[:], bo.ap().unsqueeze(0).to_broadcast((P, D)))

            # ---- K^T and V_aug persistent in SBUF (fits in bf16) ----
            ksb = kpool.tile([P, ND, S], cdt)          # K^T [p, dout_chunk, tok]
            vsb = vpool.tile([P, NK, H, hd1], cdt)     # V [tok_p, kchunk, head, 65]
            nc.vector.memset(vsb[:, :, :, hd:hd1], 1.0)  # ones columns

            # ---- x fully resident in bf16, loaded once ----
            xts = []
            for t in range(NT):
                xt = xpool.tile([P, ND, 512], cdt, tag="x")
                nc.sync.dma_start(xt[:], xT_r[:, :, ts(t, 512)])
                xts.append(xt)

            # ---- Q^T for all q512 chunks up front ----
            qsbs = []
            for qi in range(NQ):
                qsb = qpool.tile([P, ND, 512], cdt, tag="q")
                for c in range(ND):
                    wt = wpool.tile([P, ND, P], cdt, tag="w")
                    nc.sync.dma_start(wt[:], wqT_r[:, :, ts(c, P)])
                    ps = mmps.tile([P, 512], f32, tag="mm")
                    for d in range(ND):
                        mm(ps[:], wt[:, d, :], xts[qi][:, d, :],
                           d == 0, d == ND - 1)
                    nc.vector.tensor_scalar_add(qsb[:, c, :], ps[:],
                                                bqs[:, c:c + 1])
                qsbs.append(qsb)

            # ---- V then K per head-group, low head groups first so the
            #      attention for early heads can overlap late projections ----
            for g in range(NG):
                wvg = wgpool.tile([P, ND, 512], cdt, tag="wg")
                nc.sync.dma_start(wvg[:], wvT_r[:, :, ts(g, 512)])
                for t in range(NT):
                    for s in range(4):
                        kc = 4 * t + s
                        ps = mmps.tile([P, 512], f32, tag="mm")
                        for d in range(ND):
                            mm(ps[:], xts[t][:, d, ts(s, P)], wvg[:, d, :],
                               d == 0, d == ND - 1)
                        dst = vsb[:, kc, ts(g, 512 // hd), 0:hd]
                        nc.vector.tensor_tensor(
                            dst,
                            ps[:].rearrange("p (h e) -> p h e", e=hd),
                            bvb[:, ts(g, 512)].rearrange("p (h e) -> p h e", e=hd),
                            op=add)
                # K chunks covering this head group (heads 8g..8g+7)
                for c in range(4 * g, 4 * g + 4):
                    wt = wpool.tile([P, ND, P], cdt, tag="w")
                    nc.sync.dma_start(wt[:], wkT_r[:, :, ts(c, P)])
                    for t in range(NT):
                        ps = mmps.tile([P, 512], f32, tag="mm")
                        for d in range(ND):
                            mm(ps[:], wt[:, d, :], xts[t][:, d, :],
                               d == 0, d == ND - 1)
                        nc.vector.tensor_scalar_add(ksb[:, c, ts(t, 512)],
                                                    ps[:], bks[:, c:c + 1])

            # ---- attention per q512, head-PAIR inner (row-group packed
            #      logits); o-proj(qi) emitted right after its last pair so
            #      it fills PE gaps during qi+1's ACT-paced attention ----
            for qi in range(NQ):
                valsb = valspool.tile([P, ND, 512], cdt, tag="vals")
                for p in range(H // 2):
                    # heads (2p, 2p+1) live at partition offsets (0, 64) of
                    # Q/K chunk p; their K=64 logits matmuls pack into
                    # different PE row groups and run concurrently.
                    pvs = [mmps.tile([hd1, 512], f32, tag="mm",
                                     name=f"pv{p}_{qi}_{j}") for j in range(2)]
                    for kc in range(NK):
                        lg = lgps.tile([P, 2, 512], f32, tag="lg")
                        for j in range(2):
                            off = j * hd
                            mm(lg[:, j, :], ksb[off:off + hd, p, ts(kc, P)],
                               qsbs[qi][off:off + hd, p, :], True, True)
                        pt = ptpool.tile([P, 2, 512], cdt, tag="pt")
                        nc.scalar.activation(pt[:], lg[:], Exp, scale=scale)
                        for j in range(2):
                            mm(pvs[j][:], vsb[:, kc, 2 * p + j, :], pt[:, j, :],
                               kc == 0, kc == NK - 1)
                    for j in range(2):
                        off = j * hd
                        linv = lpool.tile([1, 512], f32, tag="linv")
                        nc.vector.reciprocal(linv[:], pvs[j][hd:hd1, :])
                        lbc = lpool.tile([hd, 512], f32, tag="lbc")
                        nc.gpsimd.partition_broadcast(lbc[:], linv[0:1, :])
                        nc.vector.tensor_tensor(
                            valsb[off:off + hd, p, :], pvs[j][0:hd, :],
                            lbc[:], op=mult)

                # o projection for this q512
                for g in range(NG):
                    owg = wgpool.tile([P, ND, 512], cdt, tag="wg")
                    nc.sync.dma_start(owg[:], owT_r[:, :, ts(g, 512)])
                    for s in range(4):
                        ps = mmps.tile([P, 512], f32, tag="mm")
                        for d in range(ND):
                            mm(ps[:], valsb[:, d, ts(s, P)], owg[:, d, :],
                               d == 0, d == ND - 1)
                        osb = opool.tile([P, 512], f32, tag="o")
                        nc.vector.tensor_tensor(osb[:], ps[:],
                                                bob[:, ts(g, 512)], op=add)
                        nc.sync.dma_start(
                            out.ap()[qi * 512 + s * P: qi * 512 + (s + 1) * P,
                                     ts(g, 512)],
                            osb[:])

    nc.compile()
    return nc


def _get_nc(S, D, H, SQ, use_bf16=True):
    key = (S, D, H, SQ, use_bf16)
    if key not in _NC_CACHE:
        _NC_CACHE[key] = _build_nc(S, D, H, SQ, use_bf16)
    return _NC_CACHE[key]


def _host_prep_weights(qkv_w, qkv_b, o_w, o_b, H, use_bf16=True):
    """Reorder qkv into head-major q/k/v blocks and pre-transpose."""
    import ml_dtypes
    wdt = ml_dtypes.bfloat16 if use_bf16 else np.float32
    D = o_w.shape[0]
    hd = D // H
    qkv3 = qkv_w.reshape(H, 3, hd, D)
    b3 = qkv_b.reshape(H, 3, hd)
    wqT = np.ascontiguousarray(qkv3[:, 0].reshape(D, D).T.astype(wdt))
    wkT = np.ascontiguousarray(qkv3[:, 1].reshape(D, D).T.astype(wdt))
    wvT = np.ascontiguousarray(qkv3[:, 2].reshape(D, D).T.astype(wdt))
    owT = np.ascontiguousarray(o_w.T.astype(wdt))
    return dict(
        wqT=wqT, wkT=wkT, wvT=wvT, owT=owT,
        bq=np.ascontiguousarray(b3[:, 0].reshape(D)),
        bk=np.ascontiguousarray(b3[:, 1].reshape(D)),
        bv=np.ascontiguousarray(b3[:, 2].reshape(D)),
        bo=np.ascontiguousarray(o_b),
    )


def kernel(x, qkv_w, qkv_b, o_w, o_b, _trace=False):
    from concourse.bass_utils import run_bass_kernel_spmd

    x = np.asarray(x, dtype=np.float32)
    qkv_w = np.asarray(qkv_w, dtype=np.float32)
    qkv_b = np.asarray(qkv_b, dtype=np.float32)
    o_w = np.asarray(o_w, dtype=np.float32)
    o_b = np.asarray(o_b, dtype=np.float32)

    B, S, D = x.shape
    H = 16
    n_cores = 8
    halves = n_cores // B           # 2 query-token halves per batch
    SQ = S // halves                # 1024 query tokens per core

    nc = _get_nc(S, D, H, SQ)
    shared = _host_prep_weights(qkv_w, qkv_b, o_w, o_b, H)

    in_maps = []
    for c in range(n_cores):
        b, half = divmod(c, halves)
        # this core's query tokens first; key/value order is irrelevant
        xp = np.concatenate([x[b, half * SQ:(half + 1) * SQ],
                             np.concatenate([x[b, :half * SQ],
                                             x[b, (half + 1) * SQ:]], axis=0)],
                            axis=0)
        m = dict(shared)
        import ml_dtypes
        m["xT"] = np.ascontiguousarray(xp.T.astype(ml_dtypes.bfloat16))
        in_maps.append(m)

    res = run_bass_kernel_spmd(nc, in_maps, list(range(n_cores)),
                               trace=_trace)

    out = np.empty((B, S, D), dtype=np.float32)
    for c in range(n_cores):
        b, half = divmod(c, halves)
        out[b, half * SQ:(half + 1) * SQ] = res.results[c]["out"]
    if _trace:
        return out, res
    return out



# revision 30
# speedup vs baseline: 1.1674x; 1.1674x over previous
"""Distributed MHA kernel for one TRN2 chip (8 NeuronCores), Bass/Tile.

Problem: B=4, S=2048, D=1024, H=16 full multi-head attention
(qkv proj -> scaled dot product softmax attention -> o proj).

Sharding (no collectives): core c handles batch b=c//2 and query-token
half c%2 (1024 query tokens).  Each core recomputes K/V projections for
the full 2048 tokens of its batch (zero cross-core sync).  The host
permutes x[b] so the core's query tokens come first; softmax over keys
is permutation invariant, so K/V token order doesn't matter.

v2: the QKV projections run as 3-term split-fp8 DoubleRow matmuls
(W ~= Whi + Wlo, x ~= xhi + xlo in e4m3;  W@x ~= Whi@xhi + Whi@xlo
+ Wlo@xhi, residual error ~0.1%).  DoubleRow contracts 256 din per
instruction at 0.5 cycles/row, so each projection costs 0.75x its bf16
version and the PE frees up enough to hide fully under the ACT-paced
(exp) attention phase.  Attention itself stays bf16: fp8 logits/P/V
fail the 2e-2 gate (sharp softmax rows keep elementwise quant noise).

Projection emission is interleaved with the attention pair loop so the
ACT engine starts exp'ing ~35us in while the PE finishes K/V.
"""

import numpy as np

_NC_CACHE = {}


def _build_nc(S, D, H, SQ):
    import concourse.bass as bass
    import concourse.mybir as mybir
    import concourse.tile as tile
    from concourse import bacc
    from concourse.bass import ts

    f32 = mybir.dt.float32
    bf = mybir.dt.bfloat16
    f16 = mybir.dt.float16
    e4 = mybir.dt.float8e4
    Exp = mybir.ActivationFunctionType.Exp
    add = mybir.AluOpType.add
    mult = mybir.AluOpType.mult
    sub = mybir.AluOpType.subtract
    DR = mybir.MatmulPerfMode.DoubleRow

    P = 128
    hd = D // H            # 64 head dim
    hd1 = hd + 1           # 65: V block + ones column
    ND = D // P            # 8 dout chunks
    NC = D // 256          # 4 din DoubleRow pairs
    NT = S // 512          # 4 tok512 chunks (K/V)
    NQ = SQ // 512         # 2 q512 chunks
    NK = S // P            # 16 k-token chunks
    NG = D // 512          # 2 dv512 groups
    WS_INV = 1.0 / 32.0    # weights pre-scaled x32 for fp8; undone here
    scale = 1.0 / float(np.sqrt(hd))

    nc = bacc.Bacc(trn_type="TRN2", debug=False)

    # x and weights in DoubleRow din layout [p, pair, slot, cols]:
    # din d = 256*pair + 128*slot + p
    xhi = nc.declare_dram_parameter("xhi", [P, NC, 2, S], e4, isOutput=False)
    xlo = nc.declare_dram_parameter("xlo", [P, NC, 2, S], e4, isOutput=False)
    wts = {}
    for w in ("wq", "wk", "wv"):
        for part in ("hi", "lo"):
            name = f"{w}{part}"
            wts[name] = nc.declare_dram_parameter(
                name, [P, NC, 2, D], e4, isOutput=False)
    for part in ("hi", "lo"):
        wts[f"ow{part}"] = nc.declare_dram_parameter(
            f"ow{part}", [P, NC, 2, D], e4, isOutput=False)
    bq = nc.declare_dram_parameter("bq", [D], f32, isOutput=False)
    bk = nc.declare_dram_parameter("bk", [D], f32, isOutput=False)
    bv = nc.declare_dram_parameter("bv", [D], f32, isOutput=False)
    bo = nc.declare_dram_parameter("bo", [D], f32, isOutput=False)
    out = nc.declare_dram_parameter("out", [SQ, D], f32, isOutput=True)

    def mm(ps, lhsT, rhs, start, stop):
        nc.tensor.matmul(ps, lhsT, rhs, start=start, stop=stop)

    def mm8(ps, lhsT, rhs, start, stop):
        nc.tensor.matmul(ps, lhsT, rhs, start=start, stop=stop,
                         perf_mode=DR)

    with tile.TileContext(nc) as tc:
        with (
            tc.tile_pool(name="const", bufs=1) as constp,
            tc.tile_pool(name="xpool", bufs=1) as xpool,
            tc.tile_pool(name="kpool", bufs=1) as kpool,
            tc.tile_pool(name="vpool", bufs=1) as vpool,
            tc.tile_pool(name="qpool", bufs=1) as qpool,
            tc.tile_pool(name="wpool", bufs=4) as wpool,
            tc.tile_pool(name="wgpool", bufs=2) as wgpool,
            tc.tile_pool(name="valspool", bufs=2) as valspool,
            tc.tile_pool(name="ptpool", bufs=4) as ptpool,
            tc.tile_pool(name="opool", bufs=3) as opool,
            tc.tile_pool(name="lpool", bufs=2) as lpool,
            tc.tile_pool(name="lgps", bufs=2, space="PSUM") as lgps,
            tc.tile_pool(name="mmps", bufs=2, space="PSUM") as mmps,
            tc.tile_pool(name="pvps", bufs=2, space="PSUM") as pvps,
        ):
            # ---- x resident as fp8 hi/lo splits, staged so the first
            #      projection pieces start ~3us in ----
            xh = xpool.tile([P, NC, 2, S], e4, tag="xhi")
            xl = xpool.tile([P, NC, 2, S], e4, tag="xlo")

            # ---- constants: biases (DMAs deferred past first weights) ----
            bqs = constp.tile([P, ND], f32, tag="bq")
            bks = constp.tile([P, ND], f32, tag="bk")
            bvb = constp.tile([P, D], f32, tag="bv")
            bob = constp.tile([P, D], f32, tag="bo")

            def late_loads():
                nc.sync.dma_start(bqs[:], bq.ap().rearrange("(c p) -> p c", p=P))
                nc.sync.dma_start(bks[:], bk.ap().rearrange("(c p) -> p c", p=P))
                nc.sync.dma_start(bvb[:],
                                  bv.ap().unsqueeze(0).to_broadcast((P, D)))
                nc.sync.dma_start(xh[:, :, :, SQ:S], xhi.ap()[:, :, :, SQ:S])
                nc.sync.dma_start(xl[:, :, :, SQ:S], xlo.ap()[:, :, :, SQ:S])
                nc.sync.dma_start(bob[:],
                                  bo.ap().unsqueeze(0).to_broadcast((P, D)))

            # ---- persistent K^T / V / Q^T in bf16 ----
            ksb = kpool.tile([P, ND, S], f16)          # K^T [2head, pair, tok]
            vsb = vpool.tile([P, NK, H, hd1], bf)     # V [tok_p, kc, head, 65]
            nc.vector.memset(vsb[:, :, :, hd:hd1], 0.125)  # 1/8 columns: vals x8
            qsb = qpool.tile([P, ND, SQ], f16)         # Q^T

            def proj_mm(ps, whi, wlo, xslice):
                """12 DoubleRow matmuls: Whi@xhi + Whi@xlo + Wlo@xhi."""
                first = True
                for wt, xt in ((whi, xh), (whi, xl), (wlo, xh)):
                    for c in range(NC):
                        mm8(ps[:], wt[:, c, :, :], xt[:, c, :, xslice],
                            first, (wt is wlo) and c == NC - 1)
                        first = False

            def _wload(wname, d, tag):
                nb = 2 if tag == "qw" else 4
                whi = wpool.tile([P, NC, 2, P], e4, tag=f"{tag}h",
                                 name=f"{tag}h{d}", bufs=nb)
                nc.sync.dma_start(whi[:], wts[f"{wname}hi"].ap()[:, :, :, ts(d, P)])
                wlo = wpool.tile([P, NC, 2, P], e4, tag=f"{tag}l",
                                 name=f"{tag}l{d}")
                nc.sync.dma_start(wlo[:], wts[f"{wname}lo"].ap()[:, :, :, ts(d, P)])
                return whi, wlo

            kw_cache = {}

            def q_piece(d, qi, w=None):
                # Q^T chunk d for q512 chunk qi (reloads weights per piece)
                whi, wlo = w if w is not None else _wload("wq", d, "qw")
                ps = mmps.tile([P, 512], f32, tag="mm")
                proj_mm(ps, whi, wlo, ts(qi, 512))
                nc.vector.tensor_scalar(qsb[:, d, ts(qi, 512)], ps[:],
                                        WS_INV, bqs[:, d:d + 1],
                                        mult, add)

            def k_piece(d, t):
                # K^T chunk d (heads 2d,2d+1), token block t
                if d not in kw_cache:
                    kw_cache[d] = _wload("wk", d, "kw")
                whi, wlo = kw_cache[d]
                ps = mmps.tile([P, 512], f32, tag="mm")
                proj_mm(ps, whi, wlo, ts(t, 512))
                nc.vector.tensor_scalar(ksb[:, d, ts(t, 512)], ps[:],
                                        WS_INV, bks[:, d:d + 1],
                                        mult, add)

            vw_cache = {}
            ow_cache = {}

            def v_piece(g, kc):
                # V dv-group g (heads 8g..8g+7), k-token chunk kc
                if g not in vw_cache:
                    pair = []
                    for part in ("hi", "lo"):
                        w = wgpool.tile([P, NC, 2, 512], e4, tag=f"vw{part}",
                                        name=f"vw{part}{g}")
                        nc.sync.dma_start(
                            w[:], wts[f"wv{part}"].ap()[:, :, :, ts(g, 512)])
                        pair.append(w)
                    vw_cache[g] = pair
                whi, wlo = vw_cache[g]
                ps = mmps.tile([P, 512], f32, tag="mm")
                first = True
                for wt, xt in ((whi, xh), (whi, xl), (wlo, xh)):
                    for c in range(NC):
                        mm8(ps[:], xt[:, c, :, ts(kc, P)], wt[:, c, :, :],
                            first, (wt is wlo) and c == NC - 1)
                        first = False
                dst = vsb[:, kc, ts(g, 512 // hd), 0:hd]
                nc.vector.scalar_tensor_tensor(
                    dst,
                    ps[:].rearrange("p (h e) -> p h e", e=hd),
                    WS_INV,
                    bvb[:, ts(g, 512)].rearrange("p (h e) -> p h e", e=hd),
                    op0=mult, op1=add)

            def o_piece(qi, g, si):
                # out rows [qi*512+si*128 ...], e-group g; 3-term DR
                if g not in ow_cache:
                    pair = []
                    for part in ("hi", "lo"):
                        w = wgpool.tile([P, NC, 2, 512], e4, tag=f"ow{part}",
                                        name=f"ow{part}{g}")
                        nc.sync.dma_start(
                            w[:], wts[f"ow{part}"].ap()[:, :, :, ts(g, 512)])
                        pair.append(w)
                    ow_cache[g] = pair
                owhi, owlo = ow_cache[g]
                ps = mmps.tile([P, 512], f32, tag="mm")
                first = True
                for c in range(NC):
                    for wt, part in ((owhi, 0), (owhi, 1), (owlo, 0)):
                        vt = valsbs[qi][c][part]
                        mm8(ps[:], vt[:, :, ts(si, P)], wt[:, c, :, :],
                            first, c == NC - 1 and wt is owlo)
                        first = False
                osb = opool.tile([P, 512], f32, tag="o")
                nc.vector.scalar_tensor_tensor(osb[:], ps[:], 1.0 / 256.0,
                                               bob[:, ts(g, 512)],
                                               op0=mult, op1=add)
                nc.sync.dma_start(
                    out.ap()[qi * 512 + si * P: qi * 512 + (si + 1) * P,
                             ts(g, 512)],
                    osb[:])

            def attn_pair(qi, p, fillers):
                # heads (2p, 2p+1) at partition offsets (0, 64); fillers
                # is a dict slot->emit_fn popped between exp and PV so the
                # PE never head-of-line blocks on the ACT-paced exp.
                vhi, vlo = valsbs[qi][p // 2]
                pd = p % 2
                pvs = [pvps.tile([hd1, 512], f32, tag="pv",
                                 name=f"pv{p}_{qi}_{j}") for j in range(2)]
                for kc in range(NK):
                    lg = lgps.tile([P, 2, 512], f32, tag="lg")
                    for j in range(2):
                        off = j * hd
                        mm(lg[:, j, :], ksb[off:off + hd, p, ts(kc, P)],
                           qsb[off:off + hd, p, ts(qi, 512)], True, True)
                    pt = ptpool.tile([P, 2, 512], bf, tag="pt")
                    nc.scalar.activation(pt[:], lg[:], Exp, scale=scale)
                    if kc in fillers:
                        for fn in fillers[kc]:
                            fn()
                    for j in range(2):
                        mm(pvs[j][:], vsb[:, kc, 2 * p + j, :], pt[:, j, :],
                           kc == 0, kc == NK - 1)
                pvc = []
                with tc.high_priority(offset=3000):
                    for j in range(2):
                        c = lpool.tile([hd1, 512], f32, tag="pvc")
                        nc.vector.tensor_copy(c[:], pvs[j][:])  # frees psum
                        pvc.append(c)
                for j in range(2):
                    off = j * hd
                    linv = lpool.tile([1, 512], f32, tag="linv", bufs=1)
                    nc.vector.reciprocal(linv[:], pvc[j][hd:hd1, :])
                    lbc = lpool.tile([hd, 512], f32, tag="lbc", bufs=1)
                    nc.gpsimd.partition_broadcast(lbc[:], linv[0:1, :])
                    vtmp = lpool.tile([P, 512], bf, tag="vtmp")
                    vts = vtmp[off:off + hd, :]
                    nc.vector.tensor_tensor(
                        vts, pvc[j][0:hd, :], lbc[:], op=mult)
                    nc.vector.tensor_copy(vhi[off:off + hd, pd, :], vts)
                    nc.vector.tensor_tensor(
                        vlo[off:off + hd, pd, :], vts,
                        vhi[off:off + hd, pd, :], op=sub)

            valsbs = []
            for qi in range(NQ):
                percs = []
                for c in range(NC):
                    vhi = valspool.tile([P, 2, 512], e4, tag=f"valshi{c}",
                                        name=f"valshi{qi}_{c}")
                    vlo = valspool.tile([P, 2, 512], e4, tag=f"valslo{c}",
                                        name=f"valslo{qi}_{c}")
                    percs.append((vhi, vlo))
                valsbs.append(percs)

            QP = lambda d, qi: (lambda: q_piece(d, qi))
            KP = lambda d, t: (lambda: k_piece(d, t))
            VP = lambda g, kc: (lambda: v_piece(g, kc))
            OP = lambda qi, g, si: (lambda: o_piece(qi, g, si))

            # ---- emission schedule ----
            # Pair order interleaves the two q512 chunks so projection and
            # o-proj pieces can fill every ACT-paced stretch.
            qw0 = _wload("wq", 0, "qw")
            kw_cache[0] = _wload("wk", 0, "kw")
            nc.sync.dma_start(xh[:, :, :, 0:SQ], xhi.ap()[:, :, :, 0:SQ])
            nc.sync.dma_start(xl[:, :, :, 0:SQ], xlo.ap()[:, :, :, 0:SQ])
            q_piece(0, 0, w=qw0)
            k_piece(0, 0)
            k_piece(0, 1)
            late_loads()
            for kc in range(3):
                v_piece(0, kc)
            def ow_prefetch():
                for g in range(NG):
                    if g in ow_cache:
                        continue
                    pair = []
                    for part in ("hi", "lo"):
                        w = wgpool.tile([P, NC, 2, 512], e4, tag=f"ow{part}",
                                        name=f"ow{part}{g}")
                        nc.sync.dma_start(
                            w[:], wts[f"ow{part}"].ap()[:, :, :, ts(g, 512)])
                        pair.append(w)
                    ow_cache[g] = pair

            def slots(items):
                # spread items across the 16 kc slots, ~1 per slot
                return {kc: [it] for kc, it in enumerate(items)}

            sched = [
                # (qi, p, pre-list, filler items)
                (0, 0, [], [VP(0, 3), VP(0, 4), VP(0, 5), VP(0, 6), VP(0, 7),
                            VP(0, 8), VP(0, 9), KP(0, 2), VP(0, 10), VP(0, 11),
                            VP(0, 12), KP(0, 3), VP(0, 13), VP(0, 14),
                            VP(0, 15), QP(1, 0)]),
                (0, 1, [KP(1, 0)], [KP(1, 1), KP(1, 2), KP(1, 3), QP(2, 0),
                                    VP(1, 0), VP(1, 1), VP(1, 2), VP(1, 3)]),
                (0, 2, [KP(2, 0)], [KP(2, 1), KP(2, 2), KP(2, 3), QP(3, 0),
                                    VP(1, 4), VP(1, 5), VP(1, 6), VP(1, 7)]),
                (0, 3, [KP(3, 0)], [KP(3, 1), KP(3, 2), KP(3, 3), QP(0, 1),
                                    VP(1, 8), VP(1, 9), VP(1, 10), VP(1, 11)]),
                (1, 0, [ow_prefetch], [QP(1, 1), VP(1, 12), VP(1, 13),
                                       VP(1, 14), VP(1, 15), KP(4, 0)]),
                (1, 1, [], [QP(2, 1), KP(4, 1), KP(4, 2), KP(4, 3), KP(5, 0)]),
                (1, 2, [], [QP(3, 1), KP(5, 1), KP(5, 2), KP(5, 3), KP(6, 0)]),
                (1, 3, [], [KP(6, 1), KP(6, 2), KP(6, 3), KP(7, 0), QP(4, 0)]),
                (0, 4, [], [KP(7, 1), KP(7, 2), KP(7, 3), QP(5, 0)]),
                (0, 5, [], [QP(6, 0)]),
                (0, 6, [], [QP(7, 0)]),
                (0, 7, [], [QP(4, 1)]),
                (1, 4, [], [QP(5, 1), OP(0, 0, 0), OP(0, 0, 1)]),
                (1, 5, [], [QP(6, 1), OP(0, 0, 2), OP(0, 0, 3)]),
                (1, 6, [], [QP(7, 1), OP(0, 1, 0), OP(0, 1, 1)]),
                (1, 7, [], [OP(0, 1, 2), OP(0, 1, 3)]),
            ]
            for qi, p, pre, items in sched:
                for fn in pre:
                    fn()
                attn_pair(qi, p, slots(items))
            for g in range(NG):
                for si in range(4):
                    o_piece(1, g, si)

    nc.compile()
    return nc


def _get_nc(S, D, H, SQ, use_bf16=True):
    key = (S, D, H, SQ)
    if key not in _NC_CACHE:
        _NC_CACHE[key] = _build_nc(S, D, H, SQ)
    return _NC_CACHE[key]


def _split8(arr):
    import ml_dtypes
    e4 = ml_dtypes.float8_e4m3
    hi = arr.astype(e4)
    lo = (arr - hi.astype(np.float32)).astype(e4)
    return np.ascontiguousarray(hi), np.ascontiguousarray(lo)


def _dr_layout(wT, P=128):
    """[din, cols] -> [p, pair, slot, cols] with din = 256*pair+128*slot+p."""
    D2, cols = wT.shape
    return np.ascontiguousarray(
        wT.reshape(D2 // 256, 2, P, cols).transpose(2, 0, 1, 3))


def _host_prep_weights(qkv_w, qkv_b, o_w, o_b, H):
    """Head-major q/k/v blocks, pre-transposed, fp8 hi/lo splits (x32)."""
    import ml_dtypes
    D = o_w.shape[0]
    hd = D // H
    qkv3 = qkv_w.reshape(H, 3, hd, D)
    b3 = qkv_b.reshape(H, 3, hd)
    m = {}
    for i, wname in enumerate(("wq", "wk", "wv")):
        wT = np.ascontiguousarray(qkv3[:, i].reshape(D, D).T) * 32.0
        hi, lo = _split8(_dr_layout(wT))
        m[f"{wname}hi"] = hi
        m[f"{wname}lo"] = lo
    owT = np.ascontiguousarray(o_w.T) * 32.0
    hi, lo = _split8(_dr_layout(owT))
    m["owhi"] = hi
    m["owlo"] = lo
    m["bq"] = np.ascontiguousarray(b3[:, 0].reshape(D))
    m["bk"] = np.ascontiguousarray(b3[:, 1].reshape(D))
    m["bv"] = np.ascontiguousarray(b3[:, 2].reshape(D))
    m["bo"] = np.ascontiguousarray(o_b)
    return m


def kernel(x, qkv_w, qkv_b, o_w, o_b, _trace=False):
    from concourse.bass_utils import run_bass_kernel_spmd

    x = np.asarray(x, dtype=np.float32)
    qkv_w = np.asarray(qkv_w, dtype=np.float32)
    qkv_b = np.asarray(qkv_b, dtype=np.float32)
    o_w = np.asarray(o_w, dtype=np.float32)
    o_b = np.asarray(o_b, dtype=np.float32)

    B, S, D = x.shape
    H = 16
    n_cores = 8
    halves = n_cores // B           # 2 query-token halves per batch
    SQ = S // halves                # 1024 query tokens per core

    nc = _get_nc(S, D, H, SQ)
    shared = _host_prep_weights(qkv_w, qkv_b, o_w, o_b, H)

    in_maps = []
    for c in range(n_cores):
        b, half = divmod(c, halves)
        # this core's query tokens first; key/value order is irrelevant
        xp = np.concatenate([x[b, half * SQ:(half + 1) * SQ],
                             np.concatenate([x[b, :half * SQ],
                                             x[b, (half + 1) * SQ:]], axis=0)],
                            axis=0)
        hi, lo = _split8(_dr_layout(np.ascontiguousarray(xp.T)))
        m = dict(shared)
        m["xhi"] = hi
        m["xlo"] = lo
        in_maps.append(m)

    res = run_bass_kernel_spmd(nc, in_maps, list(range(n_cores)),
                               trace=_trace)

    out = np.empty((B, S, D), dtype=np.float32)
    for c in range(n_cores):
        b, half = divmod(c, halves)
        out[b, half * SQ:(half + 1) * SQ] = res.results[c]["out"]
    if _trace:
        return out, res
    return out


# revision 31
# speedup vs baseline: 1.1686x; 1.0010x over previous
"""Distributed MHA kernel for one TRN2 chip (8 NeuronCores), Bass/Tile.

Problem: B=4, S=2048, D=1024, H=16 full multi-head attention
(qkv proj -> scaled dot product softmax attention -> o proj).

Sharding (no collectives): core c handles batch b=c//2 and query-token
half c%2 (1024 query tokens).  Each core recomputes K/V projections for
the full 2048 tokens of its batch (zero cross-core sync).  The host
permutes x[b] so the core's query tokens come first; softmax over keys
is permutation invariant, so K/V token order doesn't matter.

v2: the QKV projections run as 3-term split-fp8 DoubleRow matmuls
(W ~= Whi + Wlo, x ~= xhi + xlo in e4m3;  W@x ~= Whi@xhi + Whi@xlo
+ Wlo@xhi, residual error ~0.1%).  DoubleRow contracts 256 din per
instruction at 0.5 cycles/row, so each projection costs 0.75x its bf16
version and the PE frees up enough to hide fully under the ACT-paced
(exp) attention phase.  Attention itself stays bf16: fp8 logits/P/V
fail the 2e-2 gate (sharp softmax rows keep elementwise quant noise).

Projection emission is interleaved with the attention pair loop so the
ACT engine starts exp'ing ~35us in while the PE finishes K/V.
"""

import numpy as np

_NC_CACHE = {}


def _build_nc(S, D, H, SQ):
    import concourse.bass as bass
    import concourse.mybir as mybir
    import concourse.tile as tile
    from concourse import bacc
    from concourse.bass import ts

    f32 = mybir.dt.float32
    bf = mybir.dt.bfloat16
    f16 = mybir.dt.float16
    e4 = mybir.dt.float8e4
    Exp = mybir.ActivationFunctionType.Exp
    add = mybir.AluOpType.add
    mult = mybir.AluOpType.mult
    sub = mybir.AluOpType.subtract
    DR = mybir.MatmulPerfMode.DoubleRow

    P = 128
    hd = D // H            # 64 head dim
    hd1 = hd + 1           # 65: V block + ones column
    ND = D // P            # 8 dout chunks
    NC = D // 256          # 4 din DoubleRow pairs
    NT = S // 512          # 4 tok512 chunks (K/V)
    NQ = SQ // 512         # 2 q512 chunks
    NK = S // P            # 16 k-token chunks
    NG = D // 512          # 2 dv512 groups
    WS_INV = 1.0 / 32.0    # weights pre-scaled x32 for fp8; undone here
    scale = 1.0 / float(np.sqrt(hd))

    nc = bacc.Bacc(trn_type="TRN2", debug=False)

    # x and weights in DoubleRow din layout [p, pair, slot, cols]:
    # din d = 256*pair + 128*slot + p
    xhi = nc.declare_dram_parameter("xhi", [P, NC, 2, S], e4, isOutput=False)
    xlo = nc.declare_dram_parameter("xlo", [P, NC, 2, S], e4, isOutput=False)
    wts = {}
    for w in ("wq", "wk", "wv"):
        for part in ("hi", "lo"):
            name = f"{w}{part}"
            wts[name] = nc.declare_dram_parameter(
                name, [P, NC, 2, D], e4, isOutput=False)
    for part in ("hi", "lo"):
        wts[f"ow{part}"] = nc.declare_dram_parameter(
            f"ow{part}", [P, NC, 2, D], e4, isOutput=False)
    bq = nc.declare_dram_parameter("bq", [D], f32, isOutput=False)
    bk = nc.declare_dram_parameter("bk", [D], f32, isOutput=False)
    bv = nc.declare_dram_parameter("bv", [D], f32, isOutput=False)
    bo = nc.declare_dram_parameter("bo", [D], f32, isOutput=False)
    out = nc.declare_dram_parameter("out", [SQ, D], f32, isOutput=True)

    def mm(ps, lhsT, rhs, start, stop):
        nc.tensor.matmul(ps, lhsT, rhs, start=start, stop=stop)

    def mm8(ps, lhsT, rhs, start, stop):
        nc.tensor.matmul(ps, lhsT, rhs, start=start, stop=stop,
                         perf_mode=DR)

    with tile.TileContext(nc) as tc:
        with (
            tc.tile_pool(name="const", bufs=1) as constp,
            tc.tile_pool(name="xpool", bufs=1) as xpool,
            tc.tile_pool(name="kpool", bufs=1) as kpool,
            tc.tile_pool(name="vpool", bufs=1) as vpool,
            tc.tile_pool(name="qpool", bufs=1) as qpool,
            tc.tile_pool(name="wpool", bufs=4) as wpool,
            tc.tile_pool(name="wgpool", bufs=2) as wgpool,
            tc.tile_pool(name="valspool", bufs=2) as valspool,
            tc.tile_pool(name="ptpool", bufs=4) as ptpool,
            tc.tile_pool(name="opool", bufs=3) as opool,
            tc.tile_pool(name="lpool", bufs=2) as lpool,
            tc.tile_pool(name="lgps", bufs=2, space="PSUM") as lgps,
            tc.tile_pool(name="mmps", bufs=2, space="PSUM") as mmps,
            tc.tile_pool(name="pvps", bufs=2, space="PSUM") as pvps,
        ):
            # ---- x resident as fp8 hi/lo splits, staged so the first
            #      projection pieces start ~3us in ----
            xh = xpool.tile([P, NC, 2, S], e4, tag="xhi")
            xl = xpool.tile([P, NC, 2, S], e4, tag="xlo")

            # ---- constants: biases (DMAs deferred past first weights) ----
            bqs = constp.tile([P, ND], f32, tag="bq")
            bks = constp.tile([P, ND], f32, tag="bk")
            bvb = constp.tile([P, D], f32, tag="bv")
            bob = constp.tile([P, D], f32, tag="bo")

            def late_loads():
                nc.sync.dma_start(bvb[:],
                                  bv.ap().unsqueeze(0).to_broadcast((P, D)))
                nc.sync.dma_start(xh[:, :, :, SQ:S], xhi.ap()[:, :, :, SQ:S])
                nc.sync.dma_start(xl[:, :, :, SQ:S], xlo.ap()[:, :, :, SQ:S])
                nc.sync.dma_start(bob[:],
                                  bo.ap().unsqueeze(0).to_broadcast((P, D)))

            # ---- persistent K^T / V / Q^T in bf16 ----
            ksb = kpool.tile([P, ND, S], f16)          # K^T [2head, pair, tok]
            vsb = vpool.tile([P, NK, H, hd1], bf)     # V [tok_p, kc, head, 65]
            nc.vector.memset(vsb[:, :, :, hd:hd1], 0.125)  # 1/8 columns: vals x8
            qsb = qpool.tile([P, ND, SQ], f16)         # Q^T

            def proj_mm(ps, whi, wlo, xslice):
                """12 DoubleRow matmuls: Whi@xhi + Whi@xlo + Wlo@xhi."""
                first = True
                for wt, xt in ((whi, xh), (whi, xl), (wlo, xh)):
                    for c in range(NC):
                        mm8(ps[:], wt[:, c, :, :], xt[:, c, :, xslice],
                            first, (wt is wlo) and c == NC - 1)
                        first = False

            def _wload(wname, d, tag):
                nb = 2 if tag == "qw" else 4
                whi = wpool.tile([P, NC, 2, P], e4, tag=f"{tag}h",
                                 name=f"{tag}h{d}", bufs=nb)
                nc.sync.dma_start(whi[:], wts[f"{wname}hi"].ap()[:, :, :, ts(d, P)])
                wlo = wpool.tile([P, NC, 2, P], e4, tag=f"{tag}l",
                                 name=f"{tag}l{d}")
                nc.sync.dma_start(wlo[:], wts[f"{wname}lo"].ap()[:, :, :, ts(d, P)])
                return whi, wlo

            kw_cache = {}

            def q_piece(d, qi, w=None):
                # Q^T chunk d for q512 chunk qi (reloads weights per piece)
                whi, wlo = w if w is not None else _wload("wq", d, "qw")
                ps = mmps.tile([P, 512], f32, tag="mm")
                proj_mm(ps, whi, wlo, ts(qi, 512))
                nc.vector.tensor_scalar(qsb[:, d, ts(qi, 512)], ps[:],
                                        WS_INV, bqs[:, d:d + 1],
                                        mult, add)

            def k_piece(d, t):
                # K^T chunk d (heads 2d,2d+1), token block t
                if d not in kw_cache:
                    kw_cache[d] = _wload("wk", d, "kw")
                whi, wlo = kw_cache[d]
                ps = mmps.tile([P, 512], f32, tag="mm")
                proj_mm(ps, whi, wlo, ts(t, 512))
                nc.vector.tensor_scalar(ksb[:, d, ts(t, 512)], ps[:],
                                        WS_INV, bks[:, d:d + 1],
                                        mult, add)

            vw_cache = {}
            ow_cache = {}

            def v_piece(g, kc):
                # V dv-group g (heads 8g..8g+7), k-token chunk kc
                if g not in vw_cache:
                    pair = []
                    for part in ("hi", "lo"):
                        w = wgpool.tile([P, NC, 2, 512], e4, tag=f"vw{part}",
                                        name=f"vw{part}{g}")
                        nc.sync.dma_start(
                            w[:], wts[f"wv{part}"].ap()[:, :, :, ts(g, 512)])
                        pair.append(w)
                    vw_cache[g] = pair
                whi, wlo = vw_cache[g]
                ps = mmps.tile([P, 512], f32, tag="mm")
                first = True
                for wt, xt in ((whi, xh), (whi, xl), (wlo, xh)):
                    for c in range(NC):
                        mm8(ps[:], xt[:, c, :, ts(kc, P)], wt[:, c, :, :],
                            first, (wt is wlo) and c == NC - 1)
                        first = False
                dst = vsb[:, kc, ts(g, 512 // hd), 0:hd]
                nc.vector.scalar_tensor_tensor(
                    dst,
                    ps[:].rearrange("p (h e) -> p h e", e=hd),
                    WS_INV,
                    bvb[:, ts(g, 512)].rearrange("p (h e) -> p h e", e=hd),
                    op0=mult, op1=add)

            def o_piece(qi, g, si):
                # out rows [qi*512+si*128 ...], e-group g; 3-term DR
                if g not in ow_cache:
                    pair = []
                    for part in ("hi", "lo"):
                        w = wgpool.tile([P, NC, 2, 512], e4, tag=f"ow{part}",
                                        name=f"ow{part}{g}")
                        nc.sync.dma_start(
                            w[:], wts[f"ow{part}"].ap()[:, :, :, ts(g, 512)])
                        pair.append(w)
                    ow_cache[g] = pair
                owhi, owlo = ow_cache[g]
                ps = mmps.tile([P, 512], f32, tag="mm")
                first = True
                for c in range(NC):
                    for wt, part in ((owhi, 0), (owhi, 1), (owlo, 0)):
                        vt = valsbs[qi][c][part]
                        mm8(ps[:], vt[:, :, ts(si, P)], wt[:, c, :, :],
                            first, c == NC - 1 and wt is owlo)
                        first = False
                osb = opool.tile([P, 512], f32, tag="o")
                nc.vector.scalar_tensor_tensor(osb[:], ps[:], 1.0 / 256.0,
                                               bob[:, ts(g, 512)],
                                               op0=mult, op1=add)
                nc.sync.dma_start(
                    out.ap()[qi * 512 + si * P: qi * 512 + (si + 1) * P,
                             ts(g, 512)],
                    osb[:])

            def attn_pair(qi, p, fillers):
                # heads (2p, 2p+1) at partition offsets (0, 64); fillers
                # is a dict slot->emit_fn popped between exp and PV so the
                # PE never head-of-line blocks on the ACT-paced exp.
                vhi, vlo = valsbs[qi][p // 2]
                pd = p % 2
                pvs = [pvps.tile([hd1, 512], f32, tag="pv",
                                 name=f"pv{p}_{qi}_{j}") for j in range(2)]
                for kc in range(NK):
                    lg = lgps.tile([P, 2, 512], f32, tag="lg")
                    for j in range(2):
                        off = j * hd
                        mm(lg[:, j, :], ksb[off:off + hd, p, ts(kc, P)],
                           qsb[off:off + hd, p, ts(qi, 512)], True, True)
                    pt = ptpool.tile([P, 2, 512], bf, tag="pt")
                    nc.scalar.activation(pt[:], lg[:], Exp, scale=scale)
                    if kc in fillers:
                        for fn in fillers[kc]:
                            fn()
                    for j in range(2):
                        mm(pvs[j][:], vsb[:, kc, 2 * p + j, :], pt[:, j, :],
                           kc == 0, kc == NK - 1)
                pvc = []
                with tc.high_priority(offset=3000):
                    for j in range(2):
                        c = lpool.tile([hd1, 512], f32, tag="pvc")
                        nc.vector.tensor_copy(c[:], pvs[j][:])  # frees psum
                        pvc.append(c)
                for j in range(2):
                    off = j * hd
                    linv = lpool.tile([1, 512], f32, tag="linv", bufs=1)
                    nc.vector.reciprocal(linv[:], pvc[j][hd:hd1, :])
                    lbc = lpool.tile([hd, 512], f32, tag="lbc", bufs=1)
                    nc.gpsimd.partition_broadcast(lbc[:], linv[0:1, :])
                    vtmp = lpool.tile([P, 512], bf, tag="vtmp")
                    vts = vtmp[off:off + hd, :]
                    nc.vector.tensor_tensor(
                        vts, pvc[j][0:hd, :], lbc[:], op=mult)
                    nc.vector.tensor_copy(vhi[off:off + hd, pd, :], vts)
                    nc.vector.tensor_tensor(
                        vlo[off:off + hd, pd, :], vts,
                        vhi[off:off + hd, pd, :], op=sub)

            valsbs = []
            for qi in range(NQ):
                percs = []
                for c in range(NC):
                    vhi = valspool.tile([P, 2, 512], e4, tag=f"valshi{c}",
                                        name=f"valshi{qi}_{c}")
                    vlo = valspool.tile([P, 2, 512], e4, tag=f"valslo{c}",
                                        name=f"valslo{qi}_{c}")
                    percs.append((vhi, vlo))
                valsbs.append(percs)

            QP = lambda d, qi: (lambda: q_piece(d, qi))
            KP = lambda d, t: (lambda: k_piece(d, t))
            VP = lambda g, kc: (lambda: v_piece(g, kc))
            OP = lambda qi, g, si: (lambda: o_piece(qi, g, si))

            # ---- emission schedule ----
            # Pair order interleaves the two q512 chunks so projection and
            # o-proj pieces can fill every ACT-paced stretch.
            qw0 = _wload("wq", 0, "qw")
            kw_cache[0] = _wload("wk", 0, "kw")
            nc.sync.dma_start(bqs[:], bq.ap().rearrange("(c p) -> p c", p=P))
            nc.sync.dma_start(bks[:], bk.ap().rearrange("(c p) -> p c", p=P))
            nc.sync.dma_start(xh[:, :, :, 0:SQ], xhi.ap()[:, :, :, 0:SQ])
            nc.sync.dma_start(xl[:, :, :, 0:SQ], xlo.ap()[:, :, :, 0:SQ])
            q_piece(0, 0, w=qw0)
            k_piece(0, 0)
            k_piece(0, 1)
            late_loads()
            for kc in range(3):
                v_piece(0, kc)
            def ow_prefetch():
                for g in range(NG):
                    if g in ow_cache:
                        continue
                    pair = []
                    for part in ("hi", "lo"):
                        w = wgpool.tile([P, NC, 2, 512], e4, tag=f"ow{part}",
                                        name=f"ow{part}{g}")
                        nc.sync.dma_start(
                            w[:], wts[f"ow{part}"].ap()[:, :, :, ts(g, 512)])
                        pair.append(w)
                    ow_cache[g] = pair

            def slots(items):
                # spread items across the 16 kc slots, ~1 per slot
                return {kc: [it] for kc, it in enumerate(items)}

            sched = [
                # (qi, p, pre-list, filler items)
                (0, 0, [], [VP(0, 3), VP(0, 4), VP(0, 5), VP(0, 6), VP(0, 7),
                            VP(0, 8), VP(0, 9), KP(0, 2), VP(0, 10), VP(0, 11),
                            VP(0, 12), KP(0, 3), VP(0, 13), VP(0, 14),
                            VP(0, 15), QP(1, 0)]),
                (0, 1, [KP(1, 0)], [KP(1, 1), KP(1, 2), KP(1, 3), QP(2, 0),
                                    VP(1, 0), VP(1, 1), VP(1, 2), VP(1, 3)]),
                (0, 2, [KP(2, 0)], [KP(2, 1), KP(2, 2), KP(2, 3), QP(3, 0),
                                    VP(1, 4), VP(1, 5), VP(1, 6), VP(1, 7)]),
                (0, 3, [KP(3, 0)], [KP(3, 1), KP(3, 2), KP(3, 3), QP(0, 1),
                                    VP(1, 8), VP(1, 9), VP(1, 10), VP(1, 11)]),
                (1, 0, [ow_prefetch], [QP(1, 1), VP(1, 12), VP(1, 13),
                                       VP(1, 14), VP(1, 15), KP(4, 0)]),
                (1, 1, [], [QP(2, 1), KP(4, 1), KP(4, 2), KP(4, 3), KP(5, 0)]),
                (1, 2, [], [QP(3, 1), KP(5, 1), KP(5, 2), KP(5, 3), KP(6, 0)]),
                (1, 3, [], [KP(6, 1), KP(6, 2), KP(6, 3), KP(7, 0), QP(4, 0)]),
                (0, 4, [], [KP(7, 1), KP(7, 2), KP(7, 3), QP(5, 0)]),
                (0, 5, [], [QP(6, 0)]),
                (0, 6, [], [QP(7, 0)]),
                (0, 7, [], [QP(4, 1)]),
                (1, 4, [], [QP(5, 1), OP(0, 0, 0), OP(0, 0, 1)]),
                (1, 5, [], [QP(6, 1), OP(0, 0, 2), OP(0, 0, 3)]),
                (1, 6, [], [QP(7, 1), OP(0, 1, 0), OP(0, 1, 1)]),
                (1, 7, [], [OP(0, 1, 2), OP(0, 1, 3)]),
            ]
            for qi, p, pre, items in sched:
                for fn in pre:
                    fn()
                attn_pair(qi, p, slots(items))
            for g in range(NG):
                for si in range(4):
                    o_piece(1, g, si)

    nc.compile()
    return nc


def _get_nc(S, D, H, SQ, use_bf16=True):
    key = (S, D, H, SQ)
    if key not in _NC_CACHE:
        _NC_CACHE[key] = _build_nc(S, D, H, SQ)
    return _NC_CACHE[key]


def _split8(arr):
    import ml_dtypes
    e4 = ml_dtypes.float8_e4m3
    hi = arr.astype(e4)
    lo = (arr - hi.astype(np.float32)).astype(e4)
    return np.ascontiguousarray(hi), np.ascontiguousarray(lo)


def _dr_layout(wT, P=128):
    """[din, cols] -> [p, pair, slot, cols] with din = 256*pair+128*slot+p."""
    D2, cols = wT.shape
    return np.ascontiguousarray(
        wT.reshape(D2 // 256, 2, P, cols).transpose(2, 0, 1, 3))


def _host_prep_weights(qkv_w, qkv_b, o_w, o_b, H):
    """Head-major q/k/v blocks, pre-transposed, fp8 hi/lo splits (x32)."""
    import ml_dtypes
    D = o_w.shape[0]
    hd = D // H
    qkv3 = qkv_w.reshape(H, 3, hd, D)
    b3 = qkv_b.reshape(H, 3, hd)
    m = {}
    for i, wname in enumerate(("wq", "wk", "wv")):
        wT = np.ascontiguousarray(qkv3[:, i].reshape(D, D).T) * 32.0
        hi, lo = _split8(_dr_layout(wT))
        m[f"{wname}hi"] = hi
        m[f"{wname}lo"] = lo
    owT = np.ascontiguousarray(o_w.T) * 32.0
    hi, lo = _split8(_dr_layout(owT))
    m["owhi"] = hi
    m["owlo"] = lo
    m["bq"] = np.ascontiguousarray(b3[:, 0].reshape(D))
    m["bk"] = np.ascontiguousarray(b3[:, 1].reshape(D))
    m["bv"] = np.ascontiguousarray(b3[:, 2].reshape(D))
    m["bo"] = np.ascontiguousarray(o_b)
    return m


def kernel(x, qkv_w, qkv_b, o_w, o_b, _trace=False):
    from concourse.bass_utils import run_bass_kernel_spmd

    x = np.asarray(x, dtype=np.float32)
    qkv_w = np.asarray(qkv_w, dtype=np.float32)
    qkv_b = np.asarray(qkv_b, dtype=np.float32)
    o_w = np.asarray(o_w, dtype=np.float32)
    o_b = np.asarray(o_b, dtype=np.float32)

    B, S, D = x.shape
    H = 16
    n_cores = 8
    halves = n_cores // B           # 2 query-token halves per batch
    SQ = S // halves                # 1024 query tokens per core

    nc = _get_nc(S, D, H, SQ)
    shared = _host_prep_weights(qkv_w, qkv_b, o_w, o_b, H)

    in_maps = []
    for c in range(n_cores):
        b, half = divmod(c, halves)
        # this core's query tokens first; key/value order is irrelevant
        xp = np.concatenate([x[b, half * SQ:(half + 1) * SQ],
                             np.concatenate([x[b, :half * SQ],
                                             x[b, (half + 1) * SQ:]], axis=0)],
                            axis=0)
        hi, lo = _split8(_dr_layout(np.ascontiguousarray(xp.T)))
        m = dict(shared)
        m["xhi"] = hi
        m["xlo"] = lo
        in_maps.append(m)

    res = run_bass_kernel_spmd(nc, in_maps, list(range(n_cores)),
                               trace=_trace)

    out = np.empty((B, S, D), dtype=np.float32)
    for c in range(n_cores):
        b, half = divmod(c, halves)
        out[b, half * SQ:(half + 1) * SQ] = res.results[c]["out"]
    if _trace:
        return out, res
    return out


# revision 33
# speedup vs baseline: 1.1826x; 1.0120x over previous
"""Distributed MHA kernel for one TRN2 chip (8 NeuronCores), Bass/Tile.

Problem: B=4, S=2048, D=1024, H=16 full multi-head attention
(qkv proj -> scaled dot product softmax attention -> o proj).

Sharding (no collectives): core c handles batch b=c//2 and query-token
half c%2 (1024 query tokens).  Each core recomputes K/V projections for
the full 2048 tokens of its batch (zero cross-core sync).  The host
permutes x[b] so the core's query tokens come first; softmax over keys
is permutation invariant, so K/V token order doesn't matter.

v2: all four projections (QKV and O) run as 3-term split-fp8
DoubleRow matmuls (W ~= Whi + Wlo, x ~= xhi + xlo in e4m3;
W@x ~= Whi@xhi + Whi@xlo + Wlo@xhi, residual error ~0.15%).
DoubleRow contracts 256 din per instruction at 0.5 cycles/row, so each
projection costs 0.75x its bf16 version.  The attention core stays
16-bit: fp8 logits/P/V fail the 2e-2 gate on sharp softmax rows where
elementwise quant noise does not average out.  Q/K are stored fp16
(e5m10) rather than bf16 — same matmul cost, 8x less quantization
noise on the few |logit|~100 rows.  P and V stay bf16 (P overflows
fp16's range), vals are rebuilt as e4m3 hi+lo pairs for the split O.

Emission interleaves everything: pair order alternates the two q512
chunks, and per-kc "filler" slots stream projection / o-proj pieces
(12 DoubleRow matmuls each) into the exp-paced attention stretches so
the PE (the critical engine at ~348us busy) almost never idles.  The
emission order also guarantees every tile's writer precedes its
readers in program order — a bias DMA emitted after its first reader
races on real hardware (reads of uninitialized SBUF carry no
dependency) and was the source of a nondeterministic corruption bug.

"""

import numpy as np

_NC_CACHE = {}


def _build_nc(S, D, H, SQ):
    import concourse.bass as bass
    import concourse.mybir as mybir
    import concourse.tile as tile
    from concourse import bacc
    from concourse.bass import ts

    f32 = mybir.dt.float32
    bf = mybir.dt.bfloat16
    f16 = mybir.dt.float16
    e4 = mybir.dt.float8e4
    Exp = mybir.ActivationFunctionType.Exp
    add = mybir.AluOpType.add
    mult = mybir.AluOpType.mult
    sub = mybir.AluOpType.subtract
    DR = mybir.MatmulPerfMode.DoubleRow

    P = 128
    hd = D // H            # 64 head dim
    hd1 = hd + 1           # 65: V block + ones column
    ND = D // P            # 8 dout chunks
    NC = D // 256          # 4 din DoubleRow pairs
    NT = S // 512          # 4 tok512 chunks (K/V)
    NQ = SQ // 512         # 2 q512 chunks
    NK = S // P            # 16 k-token chunks
    NG = D // 512          # 2 dv512 groups
    WS_INV = 1.0 / 32.0    # weights pre-scaled x32 for fp8; undone here
    scale = 1.0 / float(np.sqrt(hd))

    nc = bacc.Bacc(trn_type="TRN2", debug=False)

    # x and weights in DoubleRow din layout [p, pair, slot, cols]:
    # din d = 256*pair + 128*slot + p
    xhi = nc.declare_dram_parameter("xhi", [P, NC, 2, S], e4, isOutput=False)
    xlo = nc.declare_dram_parameter("xlo", [P, NC, 2, S], e4, isOutput=False)
    wts = {}
    for w in ("wq", "wk", "wv"):
        for part in ("hi", "lo"):
            name = f"{w}{part}"
            wts[name] = nc.declare_dram_parameter(
                name, [P, NC, 2, D], e4, isOutput=False)
    for part in ("hi", "lo"):
        wts[f"ow{part}"] = nc.declare_dram_parameter(
            f"ow{part}", [P, NC, 2, D], e4, isOutput=False)
    bq = nc.declare_dram_parameter("bq", [D], f32, isOutput=False)
    bk = nc.declare_dram_parameter("bk", [D], f32, isOutput=False)
    bv = nc.declare_dram_parameter("bv", [D], f32, isOutput=False)
    bo = nc.declare_dram_parameter("bo", [D], f32, isOutput=False)
    out = nc.declare_dram_parameter("out", [SQ, D], f32, isOutput=True)

    def mm(ps, lhsT, rhs, start, stop):
        nc.tensor.matmul(ps, lhsT, rhs, start=start, stop=stop)

    def mm8(ps, lhsT, rhs, start, stop):
        nc.tensor.matmul(ps, lhsT, rhs, start=start, stop=stop,
                         perf_mode=DR)

    with tile.TileContext(nc) as tc:
        with (
            tc.tile_pool(name="const", bufs=1) as constp,
            tc.tile_pool(name="xpool", bufs=1) as xpool,
            tc.tile_pool(name="kpool", bufs=1) as kpool,
            tc.tile_pool(name="vpool", bufs=1) as vpool,
            tc.tile_pool(name="qpool", bufs=1) as qpool,
            tc.tile_pool(name="wpool", bufs=4) as wpool,
            tc.tile_pool(name="wgpool", bufs=2) as wgpool,
            tc.tile_pool(name="valspool", bufs=2) as valspool,
            tc.tile_pool(name="ptpool", bufs=4) as ptpool,
            tc.tile_pool(name="opool", bufs=3) as opool,
            tc.tile_pool(name="lpool", bufs=2) as lpool,
            tc.tile_pool(name="lgps", bufs=2, space="PSUM") as lgps,
            tc.tile_pool(name="mmps", bufs=2, space="PSUM") as mmps,
            tc.tile_pool(name="pvps", bufs=2, space="PSUM") as pvps,
        ):
            # ---- x resident as fp8 hi/lo splits, staged so the first
            #      projection pieces start ~3us in ----
            xh = xpool.tile([P, NC, 2, S], e4, tag="xhi")
            xl = xpool.tile([P, NC, 2, S], e4, tag="xlo")

            # ---- constants: biases (DMAs deferred past first weights) ----
            bqs = constp.tile([P, ND], f32, tag="bq")
            bks = constp.tile([P, ND], f32, tag="bk")
            bvb = constp.tile([P, D], f32, tag="bv")
            bob = constp.tile([P, D], f32, tag="bo")

            def late_loads():
                nc.sync.dma_start(bvb[:],
                                  bv.ap().unsqueeze(0).to_broadcast((P, D)))
                nc.sync.dma_start(xh[:, :, :, SQ:S], xhi.ap()[:, :, :, SQ:S])
                nc.sync.dma_start(xl[:, :, :, SQ:S], xlo.ap()[:, :, :, SQ:S])
                nc.sync.dma_start(bob[:],
                                  bo.ap().unsqueeze(0).to_broadcast((P, D)))

            # ---- persistent K^T / V / Q^T in bf16 ----
            ksb = kpool.tile([P, ND, S], f16)          # K^T [2head, pair, tok]
            vsb = vpool.tile([P, NK, H, hd1], bf)     # V [tok_p, kc, head, 65]
            nc.vector.memset(vsb[:, :, :, hd:hd1], 0.125)  # 1/8 columns: vals x8
            qsb = qpool.tile([P, ND, SQ], f16)         # Q^T

            def proj_mm(ps, whi, wlo, xslice):
                """12 DoubleRow matmuls: Whi@xhi + Whi@xlo + Wlo@xhi."""
                first = True
                for wt, xt in ((whi, xh), (whi, xl), (wlo, xh)):
                    for c in range(NC):
                        mm8(ps[:], wt[:, c, :, :], xt[:, c, :, xslice],
                            first, (wt is wlo) and c == NC - 1)
                        first = False

            def _wload(wname, d, tag):
                nb = 2 if tag == "qw" else 4
                whi = wpool.tile([P, NC, 2, P], e4, tag=f"{tag}h",
                                 name=f"{tag}h{d}", bufs=nb)
                nc.sync.dma_start(whi[:], wts[f"{wname}hi"].ap()[:, :, :, ts(d, P)])
                wlo = wpool.tile([P, NC, 2, P], e4, tag=f"{tag}l",
                                 name=f"{tag}l{d}")
                nc.sync.dma_start(wlo[:], wts[f"{wname}lo"].ap()[:, :, :, ts(d, P)])
                return whi, wlo

            kw_cache = {}

            def q_piece(d, qi, w=None):
                # Q^T chunk d for q512 chunk qi (reloads weights per piece)
                whi, wlo = w if w is not None else _wload("wq", d, "qw")
                ps = mmps.tile([P, 512], f32, tag="mm")
                proj_mm(ps, whi, wlo, ts(qi, 512))
                nc.vector.tensor_scalar(qsb[:, d, ts(qi, 512)], ps[:],
                                        WS_INV, bqs[:, d:d + 1],
                                        mult, add)

            def k_piece(d, t):
                # K^T chunk d (heads 2d,2d+1), token block t
                if d not in kw_cache:
                    kw_cache[d] = _wload("wk", d, "kw")
                whi, wlo = kw_cache[d]
                ps = mmps.tile([P, 512], f32, tag="mm")
                proj_mm(ps, whi, wlo, ts(t, 512))
                nc.vector.tensor_scalar(ksb[:, d, ts(t, 512)], ps[:],
                                        WS_INV, bks[:, d:d + 1],
                                        mult, add)

            vw_cache = {}
            ow_cache = {}

            def v_piece(g, kc):
                # V dv-group g (heads 8g..8g+7), k-token chunk kc
                if g not in vw_cache:
                    pair = []
                    for part in ("hi", "lo"):
                        w = wgpool.tile([P, NC, 2, 512], e4, tag=f"vw{part}",
                                        name=f"vw{part}{g}")
                        nc.sync.dma_start(
                            w[:], wts[f"wv{part}"].ap()[:, :, :, ts(g, 512)])
                        pair.append(w)
                    vw_cache[g] = pair
                whi, wlo = vw_cache[g]
                ps = mmps.tile([P, 512], f32, tag="mm")
                first = True
                for wt, xt in ((whi, xh), (whi, xl), (wlo, xh)):
                    for c in range(NC):
                        mm8(ps[:], xt[:, c, :, ts(kc, P)], wt[:, c, :, :],
                            first, (wt is wlo) and c == NC - 1)
                        first = False
                dst = vsb[:, kc, ts(g, 512 // hd), 0:hd]
                nc.vector.scalar_tensor_tensor(
                    dst,
                    ps[:].rearrange("p (h e) -> p h e", e=hd),
                    WS_INV,
                    bvb[:, ts(g, 512)].rearrange("p (h e) -> p h e", e=hd),
                    op0=mult, op1=add)

            def o_piece(qi, g, si):
                # out rows [qi*512+si*128 ...], e-group g; 3-term DR
                if g not in ow_cache:
                    pair = []
                    for part in ("hi", "lo"):
                        w = wgpool.tile([P, NC, 2, 512], e4, tag=f"ow{part}",
                                        name=f"ow{part}{g}")
                        nc.sync.dma_start(
                            w[:], wts[f"ow{part}"].ap()[:, :, :, ts(g, 512)])
                        pair.append(w)
                    ow_cache[g] = pair
                owhi, owlo = ow_cache[g]
                ps = mmps.tile([P, 512], f32, tag="mm")
                first = True
                for c in range(NC):
                    for wt, part in ((owhi, 0), (owhi, 1), (owlo, 0)):
                        vt = valsbs[qi][c][part]
                        mm8(ps[:], vt[:, :, ts(si, P)], wt[:, c, :, :],
                            first, c == NC - 1 and wt is owlo)
                        first = False
                osb = opool.tile([P, 512], f32, tag="o")
                nc.vector.scalar_tensor_tensor(osb[:], ps[:], 1.0 / 256.0,
                                               bob[:, ts(g, 512)],
                                               op0=mult, op1=add)
                nc.sync.dma_start(
                    out.ap()[qi * 512 + si * P: qi * 512 + (si + 1) * P,
                             ts(g, 512)],
                    osb[:])

            def attn_pair(qi, p, fillers):
                # heads (2p, 2p+1) at partition offsets (0, 64); fillers
                # is a dict slot->emit_fn popped between exp and PV so the
                # PE never head-of-line blocks on the ACT-paced exp.
                vhi, vlo = valsbs[qi][p // 2]
                pd = p % 2
                pvs = [pvps.tile([hd1, 512], f32, tag="pv",
                                 name=f"pv{p}_{qi}_{j}") for j in range(2)]
                for kc in range(NK):
                    lg = lgps.tile([P, 2, 512], f32, tag="lg")
                    for j in range(2):
                        off = j * hd
                        mm(lg[:, j, :], ksb[off:off + hd, p, ts(kc, P)],
                           qsb[off:off + hd, p, ts(qi, 512)], True, True)
                    pt = ptpool.tile([P, 2, 512], bf, tag="pt")
                    nc.scalar.activation(pt[:], lg[:], Exp, scale=scale)
                    if kc in fillers:
                        for fn in fillers[kc]:
                            fn()
                    for j in range(2):
                        mm(pvs[j][:], vsb[:, kc, 2 * p + j, :], pt[:, j, :],
                           kc == 0, kc == NK - 1)
                pvc = []
                with tc.high_priority(offset=3000):
                    for j in range(2):
                        c = lpool.tile([hd1, 512], f32, tag="pvc")
                        nc.vector.tensor_copy(c[:], pvs[j][:])  # frees psum
                        pvc.append(c)
                for j in range(2):
                    off = j * hd
                    linv = lpool.tile([1, 512], f32, tag="linv", bufs=1)
                    nc.vector.reciprocal(linv[:], pvc[j][hd:hd1, :])
                    lbc = lpool.tile([hd, 512], f32, tag="lbc", bufs=1)
                    nc.gpsimd.partition_broadcast(lbc[:], linv[0:1, :])
                    vtmp = lpool.tile([P, 512], bf, tag="vtmp")
                    vts = vtmp[off:off + hd, :]
                    nc.vector.tensor_tensor(
                        vts, pvc[j][0:hd, :], lbc[:], op=mult)
                    nc.vector.tensor_copy(vhi[off:off + hd, pd, :], vts)
                    nc.vector.tensor_tensor(
                        vlo[off:off + hd, pd, :], vts,
                        vhi[off:off + hd, pd, :], op=sub)

            valsbs = []
            for qi in range(NQ):
                percs = []
                for c in range(NC):
                    vhi = valspool.tile([P, 2, 512], e4, tag=f"valshi{c}",
                                        name=f"valshi{qi}_{c}")
                    vlo = valspool.tile([P, 2, 512], e4, tag=f"valslo{c}",
                                        name=f"valslo{qi}_{c}")
                    percs.append((vhi, vlo))
                valsbs.append(percs)

            QP = lambda d, qi: (lambda: q_piece(d, qi))
            KP = lambda d, t: (lambda: k_piece(d, t))
            VP = lambda g, kc: (lambda: v_piece(g, kc))
            OP = lambda qi, g, si: (lambda: o_piece(qi, g, si))

            # ---- emission schedule ----
            # Pair order interleaves the two q512 chunks so projection and
            # o-proj pieces can fill every ACT-paced stretch.
            qw0 = _wload("wq", 0, "qw")
            kw_cache[0] = _wload("wk", 0, "kw")
            nc.sync.dma_start(bqs[:], bq.ap().rearrange("(c p) -> p c", p=P))
            nc.sync.dma_start(bks[:], bk.ap().rearrange("(c p) -> p c", p=P))
            nc.sync.dma_start(xh[:, :, :, 0:SQ], xhi.ap()[:, :, :, 0:SQ])
            nc.sync.dma_start(xl[:, :, :, 0:SQ], xlo.ap()[:, :, :, 0:SQ])
            q_piece(0, 0, w=qw0)
            k_piece(0, 0)
            k_piece(0, 1)
            late_loads()
            for kc in range(3):
                v_piece(0, kc)
            def ow_prefetch():
                for g in range(NG):
                    if g in ow_cache:
                        continue
                    pair = []
                    for part in ("hi", "lo"):
                        w = wgpool.tile([P, NC, 2, 512], e4, tag=f"ow{part}",
                                        name=f"ow{part}{g}")
                        nc.sync.dma_start(
                            w[:], wts[f"ow{part}"].ap()[:, :, :, ts(g, 512)])
                        pair.append(w)
                    ow_cache[g] = pair

            def slots(items):
                # spread items across the 16 kc slots, ~1 per slot
                return {kc: [it] for kc, it in enumerate(items)}

            sched = [
                # (qi, p, pre-list, filler items)
                (0, 0, [], [VP(0, 3), VP(0, 4), VP(0, 5), VP(0, 6), VP(0, 7),
                            VP(0, 8), VP(0, 9), KP(0, 2), VP(0, 10), VP(0, 11),
                            VP(0, 12), KP(0, 3), VP(0, 13), VP(0, 14),
                            VP(0, 15), QP(1, 0)]),
                (0, 1, [KP(1, 0)], [KP(1, 1), KP(1, 2), KP(1, 3), QP(2, 0),
                                    VP(1, 0), VP(1, 1), VP(1, 2), VP(1, 3)]),
                (0, 2, [KP(2, 0)], [KP(2, 1), KP(2, 2), KP(2, 3), QP(3, 0),
                                    VP(1, 4), VP(1, 5), VP(1, 6), VP(1, 7)]),
                (0, 3, [KP(3, 0)], [KP(3, 1), KP(3, 2), KP(3, 3), QP(0, 1),
                                    VP(1, 8), VP(1, 9), VP(1, 10), VP(1, 11)]),
                (1, 0, [ow_prefetch], [QP(1, 1), VP(1, 12), VP(1, 13),
                                       VP(1, 14), VP(1, 15), KP(4, 0)]),
                (1, 1, [], [QP(2, 1), KP(4, 1), KP(4, 2), KP(4, 3)]),
                (1, 2, [], [QP(3, 1), KP(5, 0), KP(5, 1)]),
                (1, 3, [], [KP(5, 2), KP(5, 3), KP(6, 0), QP(4, 0)]),
                (0, 4, [], [KP(6, 1), KP(6, 2), QP(5, 0)]),
                (0, 5, [], [KP(6, 3), KP(7, 0), QP(6, 0)]),
                (0, 6, [], [KP(7, 1), KP(7, 2), KP(7, 3), QP(7, 0)]),
                (0, 7, [], [QP(4, 1), QP(5, 1)]),
                (1, 4, [], [OP(0, 0, 0), OP(0, 0, 1)]),
                (1, 5, [], [QP(6, 1), OP(0, 0, 2), OP(0, 0, 3)]),
                (1, 6, [], [QP(7, 1), OP(0, 1, 0), OP(0, 1, 1)]),
                (1, 7, [], [OP(0, 1, 2), OP(0, 1, 3)]),
            ]
            for qi, p, pre, items in sched:
                for fn in pre:
                    fn()
                attn_pair(qi, p, slots(items))
            for g in range(NG):
                for si in range(4):
                    o_piece(1, g, si)

    nc.compile()
    return nc


def _get_nc(S, D, H, SQ, use_bf16=True):
    key = (S, D, H, SQ)
    if key not in _NC_CACHE:
        _NC_CACHE[key] = _build_nc(S, D, H, SQ)
    return _NC_CACHE[key]


def _split8(arr):
    import ml_dtypes
    e4 = ml_dtypes.float8_e4m3
    hi = arr.astype(e4)
    lo = (arr - hi.astype(np.float32)).astype(e4)
    return np.ascontiguousarray(hi), np.ascontiguousarray(lo)


def _dr_layout(wT, P=128):
    """[din, cols] -> [p, pair, slot, cols] with din = 256*pair+128*slot+p."""
    D2, cols = wT.shape
    return np.ascontiguousarray(
        wT.reshape(D2 // 256, 2, P, cols).transpose(2, 0, 1, 3))


def _host_prep_weights(qkv_w, qkv_b, o_w, o_b, H):
    """Head-major q/k/v blocks, pre-transposed, fp8 hi/lo splits (x32)."""
    import ml_dtypes
    D = o_w.shape[0]
    hd = D // H
    qkv3 = qkv_w.reshape(H, 3, hd, D)
    b3 = qkv_b.reshape(H, 3, hd)
    m = {}
    for i, wname in enumerate(("wq", "wk", "wv")):
        wT = np.ascontiguousarray(qkv3[:, i].reshape(D, D).T) * 32.0
        hi, lo = _split8(_dr_layout(wT))
        m[f"{wname}hi"] = hi
        m[f"{wname}lo"] = lo
    owT = np.ascontiguousarray(o_w.T) * 32.0
    hi, lo = _split8(_dr_layout(owT))
    m["owhi"] = hi
    m["owlo"] = lo
    m["bq"] = np.ascontiguousarray(b3[:, 0].reshape(D))
    m["bk"] = np.ascontiguousarray(b3[:, 1].reshape(D))
    m["bv"] = np.ascontiguousarray(b3[:, 2].reshape(D))
    m["bo"] = np.ascontiguousarray(o_b)
    return m


def kernel(x, qkv_w, qkv_b, o_w, o_b, _trace=False):
    from concourse.bass_utils import run_bass_kernel_spmd

    x = np.asarray(x, dtype=np.float32)
    qkv_w = np.asarray(qkv_w, dtype=np.float32)
    qkv_b = np.asarray(qkv_b, dtype=np.float32)
    o_w = np.asarray(o_w, dtype=np.float32)
    o_b = np.asarray(o_b, dtype=np.float32)

    B, S, D = x.shape
    H = 16
    n_cores = 8
    halves = n_cores // B           # 2 query-token halves per batch
    SQ = S // halves                # 1024 query tokens per core

    nc = _get_nc(S, D, H, SQ)
    shared = _host_prep_weights(qkv_w, qkv_b, o_w, o_b, H)

    in_maps = []
    for c in range(n_cores):
        b, half = divmod(c, halves)
        # this core's query tokens first; key/value order is irrelevant
        xp = np.concatenate([x[b, half * SQ:(half + 1) * SQ],
                             np.concatenate([x[b, :half * SQ],
                                             x[b, (half + 1) * SQ:]], axis=0)],
                            axis=0)
        hi, lo = _split8(_dr_layout(np.ascontiguousarray(xp.T)))
        m = dict(shared)
        m["xhi"] = hi
        m["xlo"] = lo
        in_maps.append(m)

    res = run_bass_kernel_spmd(nc, in_maps, list(range(n_cores)),
                               trace=_trace)

    out = np.empty((B, S, D), dtype=np.float32)
    for c in range(n_cores):
        b, half = divmod(c, halves)
        out[b, half * SQ:(half + 1) * SQ] = res.results[c]["out"]
    if _trace:
        return out, res
    return out
